# revision 2
# baseline (speedup 1.0000x reference)
"""NetVLAD Trainium2 Bass kernel.

Full inputs -> full output. Shards batch N=64 across 8 NeuronCores
(8 samples per core), runs one SPMD Bass/Tile kernel, gathers.

Math (per sample, x: [C=512, P=900] channel-major):
  xT     = transpose(x)       (PE transposes of the uint16 high halves
                               of f32 x = bf16 truncation, 1 cyc/row)
  ssq[p] = sum_c xT[p,c]^2    (ACT Square+accum / DVE bn_stats from the
                               evacuated SBUF copy, split for balance)
  invn   = rsqrt(ssq)         (fast-inverse-sqrt bit trick + 2 Newton
                               steps on DVE/Pool; no ACT table switch,
                               so the single exp/square table loads once)
  logits = conv_w @ x         (f32r matmuls, C-contraction)
  lT     = transpose(high halves of logits)        (bf16 truncation)
  e      = exp(lT * invn)     (pixel-major, per-partition scale, f32r)
  s[p]   = sum_k e[p,k]; rcol = 1/s
  aT     = e * (rcol*invn)    (bf16: feeds only the main matmul)
  vlad   = aT.T @ xT - (e.T @ rcol) * centroids
           (main matmul bf16*bf16; the S-sum e.T@rcol runs in f32r
            because it is scaled by ||x||~22 and sets the absolute
            error floor; centroids are pre-negated on the host so one
            DVE scalar_tensor_tensor fuses the final combine)

Emitted as a 2-stage software pipeline (stage1: load/mm1/transposes/
ssq/invn for sample s+1 ahead of stage2: softmax/mm2/store for sample
s) so no engine head-of-line blocks on a freshly computed invn. The
bf16 truncation of x, logits and aT contributes ~2e-4 absolute error
against the 2e-2-of-1e-3*scale harness gate (measured rel 1.16e-2).
Measured ~44-82us HW exec (noisy harness) vs 183us for the f32r
baseline; cost-model marginal is ~56us/repeat against a ~45us DMA
roofline (x is 14.7MB/core in f32).
"""
import numpy as np

N, C, H, W = 64, 512, 30, 30
P = H * W              # 900
K = 64
NCORES = 8
S = N // NCORES        # samples per core
CCH = 4                # channel chunks of 128
PCHUNKS = 8            # pixel chunks per sample: 7x128 + 1x4
PW = [128] * 7 + [4]
POFF = [128 * i for i in range(8)]

# which pixel chunks get their sum-of-squares on ACT (Square+accum) vs
# DVE (bn_stats + Pool fixup)
BN_SET = (2, 3, 4)

_cache = {}


def _build_module(repeat=1):
    import concourse.bacc as bacc
    import concourse.bass as bass
    import concourse.tile as tile
    import concourse.mybir as mybir

    F32 = mybir.dt.float32
    F32R = mybir.dt.float32r
    U16 = mybir.dt.uint16
    BF16 = mybir.dt.bfloat16
    I32 = mybir.dt.int32
    AX = mybir.AxisListType
    AF = mybir.ActivationFunctionType
    OP = mybir.AluOpType

    nc = bacc.Bacc("TRN2", target_bir_lowering=False, debug=False,
                   num_devices=NCORES)

    x_d = nc.dram_tensor("x", [S, C, P], F32R, kind="ExternalInput").ap()
    cwT_d = nc.dram_tensor("cwT", [C, K], F32R, kind="ExternalInput").ap()
    cen_d = nc.dram_tensor("cen", [K, C], F32, kind="ExternalInput").ap()
    idu_d = nc.dram_tensor("identu", [128, 128], U16, kind="ExternalInput").ap()
    out_d = nc.dram_tensor("vlad", [S, K, C], F32, kind="ExternalOutput").ap()

    # mm1 halves: pixels [0,448) -> logits partitions 0-63,
    #             pixels [448,900) -> partitions 64-127 (452 wide)
    HA, HB = 448, 452

    with tile.TileContext(nc) as tc:
        with (
            tc.tile_pool(name="consts", bufs=1) as consts,
            tc.tile_pool(name="xnat", bufs=4) as xnat_pool,
            tc.tile_pool(name="lsb", bufs=3) as lsb_pool,
            tc.tile_pool(name="esb", bufs=3) as esb_pool,
            tc.tile_pool(name="atp", bufs=3) as at_pool,
            tc.tile_pool(name="xtsb", bufs=20) as xtsb_pool,
            tc.tile_pool(name="sqscr", bufs=4) as sqscr_pool,
            tc.tile_pool(name="outsb", bufs=3) as outsb_pool,
            tc.tile_pool(name="pvec", bufs=4) as pvec_pool,
            tc.tile_pool(name="pslog", bufs=1, space="PSUM") as pslog,
            tc.tile_pool(name="pslogT", bufs=1, space="PSUM") as pslogT,
            tc.tile_pool(name="psxt", bufs=3, space="PSUM") as psxt,
            tc.tile_pool(name="psmain", bufs=1, space="PSUM") as psmain,
            tc.tile_pool(name="psS", bufs=1, space="PSUM") as psS,
        ):
            # ---- constants ----
            cwT = consts.tile([128, CCH, K], F32R, tag="cwT")
            nc.sync.dma_start(
                cwT[:], cwT_d.rearrange("(j i) k -> i j k", i=128))
            identu = consts.tile([128, 128], U16, tag="identu")
            nc.sync.dma_start(identu[:], idu_d)
            identb = identu[:].bitcast(BF16)
            cen = consts.tile([K, C], F32, tag="cen")
            nc.sync.dma_start(cen[:], cen_d)

            def stage1(s):
                """load, mm1, logit transpose, x transposes, ssq, invn."""
                st = {}
                # ---- load x[s] naturally: [128, chunk, pixel] ----
                xna = xnat_pool.tile([128, CCH, P], F32R, tag="xna")
                nc.sync.dma_start(
                    xna[:], x_d[s].rearrange("(j i) p -> i j p", i=128))
                xna_hi = xna[:].bitcast(BF16).rearrange(
                    "i j (p two) -> i j p two", two=2)

                # ---- per-pixel vectors [128, PCHUNKS] ----
                ssqc = pvec_pool.tile([128, PCHUNKS], F32, tag="ssqc")
                nc.gpsimd.memset(ssqc[:], 1.0)  # keep tail rows finite

                # ---- transpose x chunks (u16 high halves); ssq; evac ----
                xts = []
                bn = pvec_pool.tile([128, len(BN_SET), 6], F32, tag="bn")
                for pj in range(PCHUNKS):
                    pw, po = PW[pj], POFF[pj]
                    if pj % 2 == 0:
                        xtp = psxt.tile([128, 2, C], BF16, tag="xtp")
                    half = pj % 2
                    for j in range(CCH):
                        nc.tensor.matmul(
                            xtp[0:pw, half, 128 * j:128 * (j + 1)],
                            xna_hi[:, j, po:po + pw, 1],
                            identb,
                            is_transpose=True, skip_group_check=True)
                    # evacuate; full pairs in one copy (chunks 6/7 single:
                    # chunk 7's tail partitions in PSUM are never written)
                    if pj < 6:
                        if pj % 2 == 0:
                            xt2 = xtsb_pool.tile([128, 2, C], BF16, tag="xt2")
                            xts.append(xt2[:, 0, :])
                            xts.append(xt2[:, 1, :])
                        else:
                            nc.vector.tensor_copy(xt2[:], xtp[:])
                    else:
                        xt = xtsb_pool.tile([128, C], BF16, tag="xt")
                        nc.vector.tensor_copy(
                            xt[0:pw, :], xtp[0:pw, half, :])
                        xts.append(xt[0:128, :])
                # ssq from the evacuated SBUF copies: the PSUM bank is
                # released by the (fast) evacuation alone, so the
                # PE-transpose ring never waits on ACT/DVE stats.
                for pj in range(PCHUNKS):
                    pw = PW[pj]
                    if pj in BN_SET:
                        bi = BN_SET.index(pj)
                        nc.vector.bn_stats(bn[0:pw, bi, :], xts[pj][0:pw, :])
                    else:
                        scr = sqscr_pool.tile([128, C], BF16, tag="scr")
                        nc.scalar.activation(
                            scr[0:pw, :], xts[pj][0:pw, :], AF.Square,
                            accum_out=ssqc[0:pw, pj:pj + 1])
                # bn fixup: ssq = ctv_e + ctv_o + 256*(me^2 + mo^2)
                nbn = len(BN_SET)
                bnt = pvec_pool.tile([128, 3, nbn], F32, tag="bnt")
                nc.gpsimd.tensor_mul(
                    bnt[:, 0, :], bn[:, :, 1], bn[:, :, 1])
                nc.gpsimd.tensor_mul(
                    bnt[:, 1, :], bn[:, :, 4], bn[:, :, 4])
                nc.gpsimd.tensor_add(
                    bnt[:, 0, :], bnt[:, 0, :], bnt[:, 1, :])
                nc.gpsimd.tensor_add(
                    bnt[:, 2, :], bn[:, :, 2], bn[:, :, 5])
                b0 = BN_SET[0]
                assert BN_SET == tuple(range(b0, b0 + nbn))
                nc.gpsimd.tensor_scalar_mul(
                    bnt[:, 1, :], bnt[:, 0, :], 256.0)
                nc.gpsimd.tensor_add(
                    ssqc[:, b0:b0 + nbn], bnt[:, 1, :], bnt[:, 2, :])

                # ---- invn = rsqrt(ssqc) via Newton on Pool ----
                invn = pvec_pool.tile([128, PCHUNKS], F32, tag="invn")
                t1 = pvec_pool.tile([128, PCHUNKS], F32, tag="t1")
                # y0 bits = 0x5f3759df - (bits(x) >> 1)
                nc.vector.tensor_scalar(
                    invn[:].bitcast(I32), ssqc[:].bitcast(I32),
                    1, None, op0=OP.logical_shift_right)
                nc.gpsimd.tensor_scalar(
                    invn[:].bitcast(I32), invn[:].bitcast(I32),
                    -1, 0x5F3759DF, op0=OP.mult, op1=OP.add)
                for _ in range(2):
                    # y = y * (1.5 - 0.5 * x * y^2)
                    nc.gpsimd.tensor_mul(t1[:], invn[:], invn[:])
                    nc.gpsimd.tensor_mul(t1[:], t1[:], ssqc[:])
                    nc.gpsimd.tensor_scalar(
                        t1[:], t1[:], -0.5, 1.5, op0=OP.mult, op1=OP.add)
                    nc.gpsimd.tensor_mul(invn[:], invn[:], t1[:])

                # ---- mm1: logits [K, 900] in two banks of 450 ----
                logA = pslog.tile([K, 450], F32, tag="logA")
                logB = pslog.tile([K, 450], F32, tag="logB")
                for j in range(CCH):
                    nc.tensor.matmul(
                        logA[:], cwT[:, j, :], xna[:, j, 0:450],
                        start=(j == 0), stop=(j == CCH - 1))
                    nc.tensor.matmul(
                        logB[:], cwT[:, j, :], xna[:, j, 450:900],
                        start=(j == 0), stop=(j == CCH - 1))

                # ---- logits -> sbuf as u16 high halves (bf16 trunc) ----
                logsb = lsb_pool.tile([K, P], BF16, tag="logsb")
                lahi = logA[:].bitcast(U16).rearrange(
                    "i (p two) -> i p two", two=2)
                lbhi = logB[:].bitcast(U16).rearrange(
                    "i (p two) -> i p two", two=2)
                nc.vector.tensor_copy(
                    logsb[:, 0:450].bitcast(U16), lahi[:, :, 1])
                nc.scalar.copy(
                    logsb[:, 450:900].bitcast(U16), lbhi[:, :, 1])

                # ---- transpose logits -> logT [pixel, k] (bf16) ----
                logT = pslogT.tile([128, PCHUNKS * K], BF16, tag="logT")
                for pj in range(PCHUNKS):
                    pw, po = PW[pj], POFF[pj]
                    nc.tensor.matmul(
                        logT[0:pw, K * pj:K * (pj + 1)],
                        logsb[:, po:po + pw],
                        identb[0:K, 0:K],
                        is_transpose=True, skip_group_check=True)

                st.update(xts=xts, invn=invn, logT=logT)
                return st

            def stage2(s, st):
                """softmax, mm2, S, final combine, store."""
                xts, invn, logT = st["xts"], st["invn"], st["logT"]
                e_sb = esb_pool.tile([128, PCHUNKS * K], F32R, tag="esb")
                nc.gpsimd.memset(e_sb[:, K * (PCHUNKS - 1):].bitcast(F32), 1.0)
                for pj in range(PCHUNKS):
                    pw = PW[pj]
                    nc.scalar.activation(
                        e_sb[0:pw, K * pj:K * (pj + 1)],
                        logT[0:pw, K * pj:K * (pj + 1)],
                        AF.Exp,
                        scale=invn[0:pw, pj:pj + 1])
                # s, padded to 9 cols (the S matmul reads 2-col windows)
                scol = pvec_pool.tile([128, PCHUNKS + 1], F32, tag="scol")
                nc.gpsimd.memset(scol[:, PCHUNKS:PCHUNKS + 1], 1.0)
                nc.vector.reduce_sum(
                    scol[:, 0:PCHUNKS],
                    e_sb[:].bitcast(F32).rearrange("i (c k) -> i c k", k=K),
                    axis=AX.X)
                rcol = pvec_pool.tile([128, PCHUNKS + 1], F32R, tag="rcol")
                with nc.allow_low_precision(
                        reason="1/s feeds the f32r S matmul; error budget "
                               "analyzed (f32r keeps ~tf32 mantissa)"):
                    nc.vector.reciprocal(rcol[:], scol[:])
                # t = 1/(s*n) = rcol * invn
                tcol = pvec_pool.tile([128, PCHUNKS], F32, tag="tcol")
                nc.gpsimd.tensor_mul(
                    tcol[:], rcol[:, 0:PCHUNKS].bitcast(F32), invn[:])

                # ---- aT = e * t (bf16: main matmul only; S uses e@rcol) ----
                aT = at_pool.tile([128, PCHUNKS * K], BF16, tag="aT")
                for pj in range(PCHUNKS):
                    pw = PW[pj]
                    nc.gpsimd.tensor_scalar_mul(
                        aT[0:pw, K * pj:K * (pj + 1)],
                        e_sb[0:pw, K * pj:K * (pj + 1)].bitcast(F32),
                        tcol[0:pw, pj:pj + 1])

                # ---- mm2: main[K,C] += aT.T @ xT (bf16) ----
                # ---- S[k] = sum_p a = e.T @ rcol windows (f32r exact) ----
                main_ps = psmain.tile([K, C], F32, tag="main")
                S_ps = psS.tile([K, 2], F32, tag="Sps")
                for pj in range(PCHUNKS):
                    pw = PW[pj]
                    nc.tensor.matmul(
                        main_ps[:], aT[0:pw, K * pj:K * (pj + 1)],
                        xts[pj][0:pw, :],
                        start=(pj == 0), stop=(pj == PCHUNKS - 1))
                    nc.tensor.matmul(
                        S_ps[:], e_sb[0:pw, K * pj:K * (pj + 1)],
                        rcol[0:pw, pj:pj + 2],
                        start=(pj == 0), stop=(pj == PCHUNKS - 1))

                # ---- final: out = main + S*(-centroids) (cen pre-negated) ----
                out_sb = outsb_pool.tile([K, C], F32, tag="outsb")
                nc.vector.scalar_tensor_tensor(
                    out_sb[:], cen[:], S_ps[:, 0:1], main_ps[:],
                    op0=OP.mult, op1=OP.add)
                nc.sync.dma_start(out_d[s], out_sb[:])

            # 2-stage software pipeline: stage1(s+1) is emitted (and thus
            # prioritized) before stage2(s), so ACT never head-of-line
            # blocks on exp waiting for a freshly-computed invn.
            samples = [s for _ in range(repeat) for s in range(S)]
            pending = None
            for s in samples:
                st = stage1(s)
                if pending is not None:
                    stage2(*pending)
                pending = (s, st)
            stage2(*pending)

    nc.compile()
    return nc


def _get_nc(repeat=1):
    key = ("nc", repeat)
    if key not in _cache:
        _cache[key] = _build_module(repeat)
    return _cache[key]


def kernel(x, conv_w, centroids):
    from concourse.bass_utils import run_bass_kernel_spmd

    x = np.ascontiguousarray(np.asarray(x, dtype=np.float32))
    conv_w = np.asarray(conv_w, dtype=np.float32)
    centroids = np.asarray(centroids, dtype=np.float32)

    nc = _get_nc()
    cwT = np.ascontiguousarray(conv_w.T)           # [C, K]
    identu = np.where(np.eye(128, dtype=bool),
                      np.uint16(0x3F80), np.uint16(0)).astype(np.uint16)
    xs = x.reshape(N, C, P)

    in_maps = []
    for core in range(NCORES):
        shard = np.ascontiguousarray(xs[core * S:(core + 1) * S])
        in_maps.append({
            "x": shard, "cwT": cwT, "cen": -centroids, "identu": identu,
        })

    res = run_bass_kernel_spmd(nc, in_maps, core_ids=list(range(NCORES)))
    out = np.concatenate([r["vlad"] for r in res.results], axis=0)
    return out.reshape(N, K, C)


# revision 4
# speedup vs baseline: 2.3567x; 2.3567x over previous
"""NetVLAD Trainium2 Bass kernel.

Full inputs -> full output. Shards batch N=64 across 8 NeuronCores
(8 samples per core), runs one SPMD Bass/Tile kernel, gathers.

Math (per sample, x: [C=512, P=900] channel-major):
  xT     = transpose(x)       (PE transposes of the uint16 high halves
                               of f32 x = bf16 truncation, 1 cyc/row)
  ssq[p] = sum_c xT[p,c]^2    (ACT Square+accum / DVE bn_stats from the
                               evacuated SBUF copy, split for balance)
  invn   = rsqrt(ssq)         (fast-inverse-sqrt bit trick + 2 Newton
                               steps on DVE/Pool; no ACT table switch,
                               so the single exp/square table loads once)
  logits = conv_w @ x         (f32r matmuls, C-contraction)
  lT     = transpose(high halves of logits)        (bf16 truncation)
  e      = exp(lT * invn)     (pixel-major, per-partition scale, f32r)
  s[p]   = sum_k e[p,k]; rcol = 1/s
  aT     = e * (rcol*invn)    (bf16: feeds only the main matmul)
  vlad   = aT.T @ xT - (e.T @ rcol) * centroids
           (main matmul bf16*bf16; the S-sum e.T@rcol runs in f32r
            because it is scaled by ||x||~22 and sets the absolute
            error floor; centroids are pre-negated on the host so one
            DVE scalar_tensor_tensor fuses the final combine)

Emitted as a 2-stage software pipeline (stage1: load/mm1/transposes/
ssq/invn for sample s+1 ahead of stage2: softmax/mm2/store for sample
s) so no engine head-of-line blocks on a freshly computed invn. The
bf16 truncation of x, logits and aT contributes ~2e-4 absolute error
against the 2e-2-of-1e-3*scale harness gate (measured rel 1.16e-2).
Measured ~44-82us HW exec (noisy harness) vs 183us for the f32r
baseline; cost-model marginal is ~56us/repeat against a ~45us DMA
roofline (x is 14.7MB/core in f32).
"""
import numpy as np

N, C, H, W = 64, 512, 30, 30
P = H * W              # 900
K = 64
NCORES = 8
S = N // NCORES        # samples per core
CCH = 4                # channel chunks of 128
PCHUNKS = 8            # pixel chunks per sample: 7x128 + 1x4
PW = [128] * 7 + [4]
POFF = [128 * i for i in range(8)]

# which pixel chunks get their sum-of-squares on ACT (Square+accum) vs
# DVE (bn_stats + Pool fixup)
BN_SET = (2, 3, 4)

_cache = {}


def _build_module(repeat=1):
    import concourse.bacc as bacc
    import concourse.bass as bass
    import concourse.tile as tile
    import concourse.mybir as mybir

    F32 = mybir.dt.float32
    F32R = mybir.dt.float32r
    U16 = mybir.dt.uint16
    BF16 = mybir.dt.bfloat16
    I32 = mybir.dt.int32
    AX = mybir.AxisListType
    AF = mybir.ActivationFunctionType
    OP = mybir.AluOpType

    nc = bacc.Bacc("TRN2", target_bir_lowering=False, debug=False,
                   num_devices=NCORES)

    x_d = nc.dram_tensor("x", [S, C, P], F32R, kind="ExternalInput").ap()
    cwT_d = nc.dram_tensor("cwT", [C, K], F32R, kind="ExternalInput").ap()
    cen_d = nc.dram_tensor("cen", [K, C], F32, kind="ExternalInput").ap()
    idu_d = nc.dram_tensor("identu", [128, 128], U16, kind="ExternalInput").ap()
    out_d = nc.dram_tensor("vlad", [S, K, C], F32, kind="ExternalOutput").ap()

    # mm1 halves: pixels [0,448) -> logits partitions 0-63,
    #             pixels [448,900) -> partitions 64-127 (452 wide)
    HA, HB = 448, 452

    with tile.TileContext(nc) as tc:
        with (
            tc.tile_pool(name="consts", bufs=1) as consts,
            tc.tile_pool(name="xnat", bufs=5) as xnat_pool,
            tc.tile_pool(name="lsb", bufs=4) as lsb_pool,
            tc.tile_pool(name="esb", bufs=4) as esb_pool,
            tc.tile_pool(name="atp", bufs=4) as at_pool,
            tc.tile_pool(name="xtsb", bufs=24) as xtsb_pool,
            tc.tile_pool(name="sqscr", bufs=4) as sqscr_pool,
            tc.tile_pool(name="outsb", bufs=3) as outsb_pool,
            tc.tile_pool(name="pvec", bufs=5) as pvec_pool,
            tc.tile_pool(name="pslog", bufs=1, space="PSUM") as pslog,
            tc.tile_pool(name="pslogT", bufs=1, space="PSUM") as pslogT,
            tc.tile_pool(name="psxt", bufs=2, space="PSUM") as psxt,
            tc.tile_pool(name="psmain", bufs=2, space="PSUM") as psmain,
            tc.tile_pool(name="psS", bufs=1, space="PSUM") as psS,
        ):
            # ---- constants ----
            cwT = consts.tile([128, CCH, K], F32R, tag="cwT")
            nc.sync.dma_start(
                cwT[:], cwT_d.rearrange("(j i) k -> i j k", i=128))
            identu = consts.tile([128, 128], U16, tag="identu")
            nc.sync.dma_start(identu[:], idu_d)
            identb = identu[:].bitcast(BF16)
            cen = consts.tile([K, C], F32, tag="cen")
            nc.sync.dma_start(cen[:], cen_d)

            def stage1(s):
                """load, mm1, logit transpose, x transposes, ssq, invn."""
                st = {}
                # ---- load x[s] naturally: [128, chunk, pixel] ----
                xna = xnat_pool.tile([128, CCH, P], F32R, tag="xna")
                nc.sync.dma_start(
                    xna[:], x_d[s].rearrange("(j i) p -> i j p", i=128))
                xna_hi = xna[:].bitcast(BF16).rearrange(
                    "i j (p two) -> i j p two", two=2)

                # ---- per-pixel vectors [128, PCHUNKS] ----
                ssqc = pvec_pool.tile([128, PCHUNKS], F32, tag="ssqc")
                nc.gpsimd.memset(ssqc[:], 1.0)  # keep tail rows finite

                # ---- transpose x chunks (u16 high halves); ssq; evac ----
                xts = []
                bn = pvec_pool.tile([128, len(BN_SET), 6], F32, tag="bn")
                for pj in range(PCHUNKS):
                    pw, po = PW[pj], POFF[pj]
                    if pj % 2 == 0:
                        xtp = psxt.tile([128, 2, C], BF16, tag="xtp")
                    half = pj % 2
                    for j in range(CCH):
                        nc.tensor.matmul(
                            xtp[0:pw, half, 128 * j:128 * (j + 1)],
                            xna_hi[:, j, po:po + pw, 1],
                            identb,
                            is_transpose=True, skip_group_check=True)
                    # evacuate; full pairs in one copy (chunks 6/7 single:
                    # chunk 7's tail partitions in PSUM are never written)
                    if pj < 6:
                        if pj % 2 == 0:
                            xt2 = xtsb_pool.tile([128, 2, C], BF16, tag="xt2")
                            xts.append(xt2[:, 0, :])
                            xts.append(xt2[:, 1, :])
                        else:
                            nc.vector.tensor_copy(xt2[:], xtp[:])
                    else:
                        xt = xtsb_pool.tile([128, C], BF16, tag="xt")
                        nc.vector.tensor_copy(
                            xt[0:pw, :], xtp[0:pw, half, :])
                        xts.append(xt[0:128, :])
                # ssq from the evacuated SBUF copies: the PSUM bank is
                # released by the (fast) evacuation alone, so the
                # PE-transpose ring never waits on ACT/DVE stats.
                for pj in range(PCHUNKS):
                    pw = PW[pj]
                    if pj in BN_SET:
                        bi = BN_SET.index(pj)
                        nc.vector.bn_stats(bn[0:pw, bi, :], xts[pj][0:pw, :])
                    else:
                        scr = sqscr_pool.tile([128, C], BF16, tag="scr")
                        nc.scalar.activation(
                            scr[0:pw, :], xts[pj][0:pw, :], AF.Square,
                            accum_out=ssqc[0:pw, pj:pj + 1])
                # bn fixup: ssq = ctv_e + ctv_o + 256*(me^2 + mo^2)
                nbn = len(BN_SET)
                bnt = pvec_pool.tile([128, 3, nbn], F32, tag="bnt")
                nc.gpsimd.tensor_mul(
                    bnt[:, 0, :], bn[:, :, 1], bn[:, :, 1])
                nc.gpsimd.tensor_mul(
                    bnt[:, 1, :], bn[:, :, 4], bn[:, :, 4])
                nc.gpsimd.tensor_add(
                    bnt[:, 0, :], bnt[:, 0, :], bnt[:, 1, :])
                nc.gpsimd.tensor_add(
                    bnt[:, 2, :], bn[:, :, 2], bn[:, :, 5])
                b0 = BN_SET[0]
                assert BN_SET == tuple(range(b0, b0 + nbn))
                nc.gpsimd.tensor_scalar_mul(
                    bnt[:, 1, :], bnt[:, 0, :], 256.0)
                nc.gpsimd.tensor_add(
                    ssqc[:, b0:b0 + nbn], bnt[:, 1, :], bnt[:, 2, :])

                # ---- invn = rsqrt(ssqc) via Newton on Pool ----
                invn = pvec_pool.tile([128, PCHUNKS], F32, tag="invn")
                t1 = pvec_pool.tile([128, PCHUNKS], F32, tag="t1")
                # y0 bits = 0x5f3759df - (bits(x) >> 1)
                nc.vector.tensor_scalar(
                    invn[:].bitcast(I32), ssqc[:].bitcast(I32),
                    1, None, op0=OP.logical_shift_right)
                nc.gpsimd.tensor_scalar(
                    invn[:].bitcast(I32), invn[:].bitcast(I32),
                    -1, 0x5F3759DF, op0=OP.mult, op1=OP.add)
                for _ in range(2):
                    # y = y * (1.5 - 0.5 * x * y^2)
                    nc.gpsimd.tensor_mul(t1[:], invn[:], invn[:])
                    nc.gpsimd.tensor_mul(t1[:], t1[:], ssqc[:])
                    nc.gpsimd.tensor_scalar(
                        t1[:], t1[:], -0.5, 1.5, op0=OP.mult, op1=OP.add)
                    nc.gpsimd.tensor_mul(invn[:], invn[:], t1[:])

                # ---- mm1: logits [K, 900] in two banks of 450 ----
                logA = pslog.tile([K, 450], F32, tag="logA")
                logB = pslog.tile([K, 450], F32, tag="logB")
                for j in range(CCH):
                    nc.tensor.matmul(
                        logA[:], cwT[:, j, :], xna[:, j, 0:450],
                        start=(j == 0), stop=(j == CCH - 1))
                    nc.tensor.matmul(
                        logB[:], cwT[:, j, :], xna[:, j, 450:900],
                        start=(j == 0), stop=(j == CCH - 1))

                # ---- logits -> sbuf as u16 high halves (bf16 trunc) ----
                logsb = lsb_pool.tile([K, P], BF16, tag="logsb")
                lahi = logA[:].bitcast(U16).rearrange(
                    "i (p two) -> i p two", two=2)
                lbhi = logB[:].bitcast(U16).rearrange(
                    "i (p two) -> i p two", two=2)
                nc.vector.tensor_copy(
                    logsb[:, 0:450].bitcast(U16), lahi[:, :, 1])
                nc.scalar.copy(
                    logsb[:, 450:900].bitcast(U16), lbhi[:, :, 1])

                # ---- transpose logits -> logT [pixel, k] (bf16) ----
                logT = pslogT.tile([128, PCHUNKS * K], BF16, tag="logT")
                for pj in range(PCHUNKS):
                    pw, po = PW[pj], POFF[pj]
                    nc.tensor.matmul(
                        logT[0:pw, K * pj:K * (pj + 1)],
                        logsb[:, po:po + pw],
                        identb[0:K, 0:K],
                        is_transpose=True, skip_group_check=True)

                st.update(xts=xts, invn=invn, logT=logT)
                return st

            def stage2(s, st):
                """softmax, mm2, S, final combine, store."""
                xts, invn, logT = st["xts"], st["invn"], st["logT"]
                e_sb = esb_pool.tile([128, PCHUNKS * K], F32R, tag="esb")
                nc.gpsimd.memset(e_sb[:, K * (PCHUNKS - 1):].bitcast(F32), 1.0)
                for pj in range(PCHUNKS):
                    pw = PW[pj]
                    nc.scalar.activation(
                        e_sb[0:pw, K * pj:K * (pj + 1)],
                        logT[0:pw, K * pj:K * (pj + 1)],
                        AF.Exp,
                        scale=invn[0:pw, pj:pj + 1])
                # s, padded to 9 cols (the S matmul reads 2-col windows)
                scol = pvec_pool.tile([128, PCHUNKS + 1], F32, tag="scol")
                nc.gpsimd.memset(scol[:, PCHUNKS:PCHUNKS + 1], 1.0)
                nc.vector.reduce_sum(
                    scol[:, 0:PCHUNKS],
                    e_sb[:].bitcast(F32).rearrange("i (c k) -> i c k", k=K),
                    axis=AX.X)
                rcol = pvec_pool.tile([128, PCHUNKS + 1], F32R, tag="rcol")
                with nc.allow_low_precision(
                        reason="1/s feeds the f32r S matmul; error budget "
                               "analyzed (f32r keeps ~tf32 mantissa)"):
                    nc.vector.reciprocal(rcol[:], scol[:])
                # t = 1/(s*n) = rcol * invn
                tcol = pvec_pool.tile([128, PCHUNKS], F32, tag="tcol")
                nc.gpsimd.tensor_mul(
                    tcol[:], rcol[:, 0:PCHUNKS].bitcast(F32), invn[:])

                # ---- aT = e * t (bf16: main matmul only; S uses e@rcol) ----
                aT = at_pool.tile([128, PCHUNKS * K], BF16, tag="aT")
                for pj in range(PCHUNKS):
                    pw = PW[pj]
                    nc.gpsimd.tensor_scalar_mul(
                        aT[0:pw, K * pj:K * (pj + 1)],
                        e_sb[0:pw, K * pj:K * (pj + 1)].bitcast(F32),
                        tcol[0:pw, pj:pj + 1])

                # ---- mm2: main[K,C] += aT.T @ xT (bf16) ----
                # ---- S[k] = sum_p a = e.T @ rcol windows (f32r exact) ----
                main_ps = psmain.tile([K, C], F32, tag="main")
                S_ps = psS.tile([K, 2], F32, tag="Sps")
                for pj in range(PCHUNKS):
                    pw = PW[pj]
                    nc.tensor.matmul(
                        main_ps[:], aT[0:pw, K * pj:K * (pj + 1)],
                        xts[pj][0:pw, :],
                        start=(pj == 0), stop=(pj == PCHUNKS - 1))
                    nc.tensor.matmul(
                        S_ps[:], e_sb[0:pw, K * pj:K * (pj + 1)],
                        rcol[0:pw, pj:pj + 2],
                        start=(pj == 0), stop=(pj == PCHUNKS - 1))

                # ---- final: out = main + S*(-centroids) (cen pre-negated) ----
                out_sb = outsb_pool.tile([K, C], F32, tag="outsb")
                nc.vector.scalar_tensor_tensor(
                    out_sb[:], cen[:], S_ps[:, 0:1], main_ps[:],
                    op0=OP.mult, op1=OP.add)
                nc.sync.dma_start(out_d[s], out_sb[:])

            # 2-stage software pipeline: stage1(s+1) is emitted (and thus
            # prioritized) before stage2(s), so ACT never head-of-line
            # blocks on exp waiting for a freshly-computed invn.
            samples = [s for _ in range(repeat) for s in range(S)]
            pending = None
            for s in samples:
                st = stage1(s)
                if pending is not None:
                    stage2(*pending)
                pending = (s, st)
            stage2(*pending)

    nc.compile()
    return nc


def _get_nc(repeat=1):
    key = ("nc", repeat)
    if key not in _cache:
        _cache[key] = _build_module(repeat)
    return _cache[key]


def kernel(x, conv_w, centroids):
    from concourse.bass_utils import run_bass_kernel_spmd

    x = np.ascontiguousarray(np.asarray(x, dtype=np.float32))
    conv_w = np.asarray(conv_w, dtype=np.float32)
    centroids = np.asarray(centroids, dtype=np.float32)

    nc = _get_nc()
    cwT = np.ascontiguousarray(conv_w.T)           # [C, K]
    identu = np.where(np.eye(128, dtype=bool),
                      np.uint16(0x3F80), np.uint16(0)).astype(np.uint16)
    xs = x.reshape(N, C, P)

    in_maps = []
    for core in range(NCORES):
        shard = np.ascontiguousarray(xs[core * S:(core + 1) * S])
        in_maps.append({
            "x": shard, "cwT": cwT, "cen": -centroids, "identu": identu,
        })

    res = run_bass_kernel_spmd(nc, in_maps, core_ids=list(range(NCORES)))
    out = np.concatenate([r["vlad"] for r in res.results], axis=0)
    return out.reshape(N, K, C)
